# revision 1
# baseline (speedup 1.0000x reference)
"""DenseCRF mean-field inference on 8 Trainium2 NeuronCores.

Math: per image (1 here), 5 iterations of
    q_hat = U + 4*((q/n) @ K)/n + 2*(q @ S);  q = softmax(q_hat, axis=0)
with K[i,j] = exp(-0.5*d2(i,j)) the dense 9216x9216 bilateral kernel,
n = sqrt(K @ 1) and S the (input-independent) 71x71 spatial kernel matrix.

Key transform: fold everything iteration-invariant into one matrix
    M[i,j] = 4*K[i,j]*rn_i*rn_j + 2*S[i,j],   rn = 1/sqrt(colsum(K))
so each iteration is just q_hat = U + q @ M.  M's columns (output pixels j)
are sharded over the 8 cores: each core holds M[:, mine] = [9216, 1152]
bf16 (~21 MB) resident in SBUF.

Build (one pass over the 10.6M-entry block per core):
  1. E = exp(f.f - 0.5|f_i|^2 - 0.5|f_j|^2 + ln4) = 4*K, via a 14-row
     extended-feature matmul (PSUM) + one ACT exp directly into the
     resident bf16 tile.
  2. cs = colsum(E) = 4*colsum(K) via ones-matmuls over the bf16 tile;
     rn = 1/sqrt(colsum K) = exp(-0.5*ln(cs) + ln 2)  (Ln+Exp share one
     ACT table set); AllGather rn (37KB).
  3. In-place rescale: M = (E * rn_j) * rn_i + 2S, two fused chunk ops
     split across DVE and GPSIMD; 2S streams from a host-precomputed
     per-core [9216,1152] bf16 input (input-independent constant).

Iteration: 216 accumulating matmuls (lhsT = gathered q chunks [128,21]
bf16, rhs = M slices, PSUM [21,1152]), +U on DVE, 9 PE-transposes to
pixel-major, softmax over the 21 classes via ACT exp + accum_out +
reciprocal + scale, AllGather of the 1152x21 bf16 q shard (~48KB).

Extended-feature pairing (lhsT row c with rhs row c; sum over c = T'):
  c0-4 : (f_i, f_j)      f = [y/70, x/70, r/12, g/12, b/12]
  c5-8 : (diag_i, 1)     diag = [-(y^2+x^2)/(2*70^2), -r^2/288, ...]
  c9   : (ln4, 1)
  c10-13:(1, diag_j)
P-side rows: [f(0:5), diag(5:9), ln4(9), ones(10:14)]
Q-side rows: [f(0:5), ones(5:10), diag(10:14)]
"""

import numpy as np
import ml_dtypes

H = 96
W = 96
P = H * W            # 9216 pixels
L = 21               # classes
NCORES = 8
PSH = P // NCORES    # 1152 pixels per core
NI = P // 128        # 72 contraction chunks
NJ = PSH // 128      # 9 output-pixel chunks per core
NSLAB = 8            # lhsT feature slabs of 1152 columns
SXY_BF = 70.0
SC_BF = 12.0
LN4 = float(np.log(4.0).astype(np.float32))

_bf16 = ml_dtypes.bfloat16

_CACHE = {}
TRACE = False      # set by test harness for profiling runs
LAST_RESULT = None


# ----------------------------------------------------------------------------
# host-side input-independent constants
# ----------------------------------------------------------------------------

def _host_constants():
    if "consts" in _CACHE:
        return _CACHE["consts"]

    # 1D spatial gaussian band matrix B[a,b] = exp(-(a-b)^2/72)/z, |a-b|<=35
    sig_sq = 36.0
    rr = 35
    g1 = np.exp(-((np.arange(2 * rr + 1, dtype=np.float64) - rr) ** 2)
                / (2 * sig_sq))
    z = g1.sum()
    idx = np.arange(H)
    d = idx[:, None] - idx[None, :]
    B = np.where(np.abs(d) <= rr,
                 np.exp(-(d.astype(np.float64) ** 2) / (2 * sig_sq)) / z, 0.0)

    # S2 = 2 * kron(B, B), per-core column blocks, laid out [NI, 128, PSH] bf16
    ys = np.arange(P) // W
    xs = np.arange(P) % W
    s2_blocks = []
    for c in range(NCORES):
        jj = np.arange(c * PSH, (c + 1) * PSH)
        blk = 2.0 * B[np.ix_(ys, ys[jj])] * B[np.ix_(xs, xs[jj])]  # [P, PSH]
        s2_blocks.append(
            np.ascontiguousarray(blk.reshape(NI, 128, PSH).astype(_bf16)))

    # const rows: [y/70, x/70, -0.5*((y/70)^2+(x/70)^2), ones, zeros, ln4]
    yf = (ys / SXY_BF).astype(np.float32)
    xf = (xs / SXY_BF).astype(np.float32)
    syx = (-0.5 * (yf.astype(np.float64) ** 2 + xf.astype(np.float64) ** 2)
           ).astype(np.float32)
    cst = np.stack([yf, xf, syx,
                    np.ones(P, np.float32),
                    np.zeros(P, np.float32),
                    np.full(P, LN4, np.float32)], 0)  # [6, P]
    # collapse matrix for 4-way column-packed matmuls: C[p,l]=1 iff p%32==l
    pp = np.arange(128)
    cmat = (np.equal.outer(pp % 32, np.arange(L))).astype(_bf16)
    _CACHE["consts"] = (s2_blocks, cst, cmat)
    return _CACHE["consts"]


# ----------------------------------------------------------------------------
# device program
# ----------------------------------------------------------------------------

def _build_bass(niters=5, pack=True):
    key = ("nc", niters, pack)
    if key in _CACHE:
        return _CACHE[key]

    import concourse.bass as bass
    import concourse.bacc as bacc
    import concourse.tile as tile
    import concourse.mybir as mybir
    from concourse.masks import make_identity

    f32 = mybir.dt.float32
    bf16 = mybir.dt.bfloat16
    AF = mybir.ActivationFunctionType
    ALU = mybir.AluOpType

    nc = bacc.Bacc("TRN2", num_devices=NCORES)

    unary_m = nc.dram_tensor("unary_m", [L, PSH], f32, kind="ExternalInput")
    ref_f = nc.dram_tensor("ref_f", [3, P], f32, kind="ExternalInput")
    ref_m = nc.dram_tensor("ref_m", [3, PSH], f32, kind="ExternalInput")
    cst_f = nc.dram_tensor("cst_f", [6, P], f32, kind="ExternalInput")
    cst_m = nc.dram_tensor("cst_m", [6, PSH], f32, kind="ExternalInput")
    s2b = nc.dram_tensor("s2b", [NI, 128, PSH], bf16, kind="ExternalInput")
    cmat = nc.dram_tensor("cmat", [128, L], bf16, kind="ExternalInput")
    qout = nc.dram_tensor("qout", [NJ, 128, L], f32, kind="ExternalOutput")

    rg = [list(range(NCORES))]

    with tile.TileContext(nc) as tc:
        with tc.tile_pool(name="dram", bufs=1, space="DRAM") as dram:
            PL_d = dram.tile([14, P], f32)          # P-side operand source
            rn_in_d = dram.tile([1, PSH], f32)      # my rn, AG input
            rn_out_d = dram.tile([NI, 128], f32)    # full rn, AG output
            rnb_d = dram.tile([1, PSH], bf16)       # my rn in bf16
            qsh_d = dram.tile([NJ, 128, L], bf16)   # q shard, AG input
            qfl_d = dram.tile([NI, 128, L], bf16)   # full q, AG output

            with tc.tile_pool(name="persist", bufs=1) as persist:
                # ---- operand assembly (base-0 scratch -> DRAM -> rows) ----
                asmQR = persist.tile([14, PSH], f32)   # rhs (mine, Q-side)
                with (
                    tc.tile_pool(name="build1", bufs=1) as build1,
                    tc.tile_pool(name="scrp", bufs=1) as scrp,
                ):
                    asmPL = build1.tile([14, P], f32)  # lhsT src (full, P)

                    def make_scratch(refsrc, n, sfx):
                        rs = scrp.tile([3, n], f32, name=f"rs{sfx}")
                        nc.sync.dma_start(rs[:], refsrc[:, :])
                        nc.vector.tensor_scalar_mul(rs[:], rs[:], 1.0 / SC_BF)
                        rq = scrp.tile([3, n], f32, name=f"rq{sfx}")
                        nc.vector.tensor_mul(rq[:], rs[:], rs[:])
                        nc.vector.tensor_scalar_mul(rq[:], rq[:], -0.5)
                        rs_d = dram.tile([3, n], f32, name=f"rsd{sfx}")
                        rq_d = dram.tile([3, n], f32, name=f"rqd{sfx}")
                        nc.gpsimd.dma_start(rs_d[:], rs[:])
                        nc.gpsimd.dma_start(rq_d[:], rq[:])
                        return rs_d, rq_d

                    rsf, rqf = make_scratch(ref_f, P, "f")
                    rsm, rqm = make_scratch(ref_m, PSH, "m")

                    # P-side rows: [f(0:5), diag(5:9), ln4(9), ones(10:14)]
                    nc.sync.dma_start(asmPL[0:2, :], cst_f[0:2, :])
                    nc.sync.dma_start(asmPL[2:5, :], rsf[:])
                    nc.sync.dma_start(asmPL[5:6, :], cst_f[2:3, :])
                    nc.sync.dma_start(asmPL[6:9, :], rqf[:])
                    nc.sync.dma_start(asmPL[9:10, :], cst_f[5:6, :])
                    nc.sync.dma_start(asmPL[10:14, :],
                                      cst_f[3:4, :].to_broadcast((4, P)))
                    # Q-side rows: [f(0:5), ones(5:10), diag(10:14)]
                    nc.sync.dma_start(asmQR[0:2, :], cst_m[0:2, :])
                    nc.sync.dma_start(asmQR[2:5, :], rsm[:])
                    nc.sync.dma_start(asmQR[5:10, :],
                                      cst_m[3:4, :].to_broadcast((5, PSH)))
                    nc.sync.dma_start(asmQR[10:11, :], cst_m[2:3, :])
                    nc.sync.dma_start(asmQR[11:14, :], rqm[:])

                    nc.gpsimd.dma_start(PL_d[:, :], asmPL[:])

                # ---- persistent iteration state -----------------------
                ident = persist.tile([L, L], f32)
                make_identity(nc, ident[:])
                U_sb = persist.tile([L, PSH], f32)
                rnI = persist.tile([128, NI], f32)
                rnJb = persist.tile([128, PSH], bf16)

                # ---- M build: E = exp(T') into resident bf16 tile ---------
                mpool = tc.tile_pool(name="mres", bufs=1)
                mpool_h = mpool.__enter__()
                Mt = mpool_h.tile([128, NI, PSH], bf16, name="Mt")

                with (
                    tc.tile_pool(name="itq", bufs=1) as itq,
                    tc.tile_pool(name="ite", bufs=4) as ite,
                    tc.tile_pool(name="ittp", bufs=2, space="PSUM") as ittp,
                ):
                    # U = ln(clip(u)) (overlaps the build)
                    ut = itq.tile([L, PSH], f32, tag="qh")
                    nc.sync.dma_start(ut[:], unary_m[:, :])
                    nc.vector.tensor_scalar(ut[:], ut[:], 1e-5, 1.0,
                                            op0=ALU.max, op1=ALU.min)
                    nc.scalar.activation(U_sb[:], ut[:], AF.Ln)
                    cmt = itq.tile([128, L], bf16, name="cmt")
                    nc.sync.dma_start(cmt[:], cmat[:, :])
                    pkx = itq.tile([128, PSH], bf16, name="pkx")
                    nc.vector.memset(pkx[:], 0.0)

                    # ---- E = exp(T') into the resident bf16 tile ----------
                    with (
                        tc.tile_pool(name="slab", bufs=2) as slabp,
                        tc.tile_pool(name="eps", bufs=2, space="PSUM") as eps,
                    ):
                        for sb in range(NSLAB):
                            sl = slabp.tile([14, PSH], f32, tag="sl")
                            nc.sync.dma_start(
                                sl[:], PL_d[:, sb * PSH:(sb + 1) * PSH])
                            for k in range(NI // NSLAB):
                                ic = sb * (NI // NSLAB) + k
                                ps = eps.tile([128, PSH], f32, tag="eps")
                                lh = sl[:, k * 128:(k + 1) * 128]
                                for (o, n) in ((0, 512), (512, 512),
                                               (1024, 128)):
                                    nc.tensor.matmul(
                                        ps[:, o:o + n], lh,
                                        asmQR[:, o:o + n],
                                        start=True, stop=True)
                                nc.scalar.activation(Mt[:, ic, :], ps[:],
                                                     AF.Exp)

                    # ---- colsums (4-way column-packed) -> rn --------------
                    with (
                        tc.tile_pool(name="csp", bufs=1, space="PSUM") as csp,
                        tc.tile_pool(name="cs2p", bufs=1, space="PSUM") as c2p,
                        tc.tile_pool(name="cst1", bufs=1) as cst1,
                    ):
                        onesb = cst1.tile([128, 1], bf16)
                        nc.vector.memset(onesb[:], 1.0)
                        ones4 = cst1.tile([128, 1], f32)
                        nc.vector.memset(ones4[:], 0.0)
                        for t in range(4):
                            nc.vector.memset(ones4[32 * t:32 * t + 1, :], 1.0)
                        cs = csp.tile([128, PSH], f32)
                        for ic in range(NI):
                            t = ic % 4
                            for (o, n) in ((0, 512), (512, 512), (1024, 128)):
                                nc.tensor.matmul(cs[32 * t:32 * t + 1,
                                                    o:o + n],
                                                 onesb[:],
                                                 Mt[:, ic, o:o + n],
                                                 start=(ic < 4),
                                                 stop=(ic >= NI - 4),
                                                 tile_position=(0, 32 * t),
                                                 skip_group_check=True)
                        csx = cst1.tile([128, PSH], f32)
                        nc.vector.memset(csx[:], 0.0)
                        for t in range(4):
                            nc.vector.tensor_copy(csx[32 * t:32 * t + 1, :],
                                                  cs[32 * t:32 * t + 1, :])
                        cs2 = c2p.tile([1, PSH], f32)
                        for (o, n) in ((0, 512), (512, 512), (1024, 128)):
                            nc.tensor.matmul(cs2[:, o:o + n], ones4[:],
                                             csx[:, o:o + n],
                                             start=True, stop=True)
                        # cs2 = colsum(E) = 4*colsum(K);
                        # rn = colsum(K)^-1/2 = exp(-0.5*ln(cs2) + ln 2)
                        s_sb = cst1.tile([1, PSH], f32)
                        nc.scalar.activation(s_sb[:], cs2[:], AF.Ln)
                        nc.vector.tensor_scalar(s_sb[:], s_sb[:], -0.5,
                                                float(np.log(2.0)),
                                                op0=ALU.mult, op1=ALU.add)
                        rn_sb = cst1.tile([1, PSH], f32)
                        nc.scalar.activation(rn_sb[:], s_sb[:], AF.Exp)
                        nc.gpsimd.dma_start(rn_in_d[:, :], rn_sb[:])
                        rnb_sb = cst1.tile([1, PSH], bf16)
                        nc.vector.tensor_copy(rnb_sb[:], rn_sb[:])
                        nc.gpsimd.dma_start(rnb_d[:, :], rnb_sb[:])

                    nc.gpsimd.collective_compute(
                        "AllGather", mybir.AluOpType.bypass, replica_groups=rg,
                        ins=[rn_in_d.rearrange("a b -> (a b)")],
                        outs=[rn_out_d.rearrange("a b -> (a b)")])
                    nc.sync.dma_start(rnI[:], rn_out_d.rearrange("a b -> b a"))
                    nc.sync.dma_start(rnJb[:],
                                      rnb_d[0:1, :].to_broadcast((128, PSH)))

                    # ---- in-place rescale: M = (E*rn_i)*rn_j + 2S ---------
                    with tc.tile_pool(name="s2p", bufs=6) as s2p:
                        for ic in range(NI):
                            sc = s2p.tile([128, PSH], bf16, tag="sc")
                            nc.sync.dma_start(sc[:], s2b[ic, :, :])
                            mt_c = Mt[:, ic, :]
                            nc.vector.scalar_tensor_tensor(
                                mt_c, mt_c, rnI[:, ic:ic + 1], rnJb[:],
                                op0=ALU.mult, op1=ALU.mult)
                            if ic % 7 < 4:
                                nc.vector.tensor_add(mt_c, mt_c, sc[:])
                            else:
                                nc.gpsimd.tensor_add(mt_c, mt_c, sc[:])

                    # ---- iterations (4-way column-packed matvec) ----------
                    with (
                        tc.tile_pool(name="pk", bufs=1, space="PSUM") as pkp,
                        tc.tile_pool(name="itps", bufs=1,
                                     space="PSUM") as itps,
                    ):
                        qf_cur = None
                        for it in range(niters + 1):
                            if it == 0:
                                qh_cur = U_sb
                            else:
                                ps = itps.tile([L, PSH], f32, tag="qbps")
                                if pack:
                                    pk = pkp.tile([128, PSH], f32, tag="pk")
                                    for ic in range(NI):
                                        t = ic % 4
                                        lhq = qf_cur[:, ic, :]
                                        for (o, n) in ((0, 512), (512, 512),
                                                       (1024, 128)):
                                            nc.tensor.matmul(
                                                pk[32 * t:32 * t + L,
                                                   o:o + n],
                                                lhq, Mt[:, ic, o:o + n],
                                                start=(ic < 4),
                                                stop=(ic >= NI - 4),
                                                tile_position=(0, 32 * t),
                                                skip_group_check=True)
                                    for t in range(4):
                                        nc.vector.tensor_copy(
                                            pkx[32 * t:32 * t + L, :],
                                            pk[32 * t:32 * t + L, :])
                                    for (o, n) in ((0, 512), (512, 512),
                                                   (1024, 128)):
                                        nc.tensor.matmul(ps[:, o:o + n],
                                                         cmt[:],
                                                         pkx[:, o:o + n],
                                                         start=True,
                                                         stop=True)
                                else:
                                    for ic in range(NI):
                                        lhq = qf_cur[:, ic, :]
                                        for (o, n) in ((0, 512), (512, 512),
                                                       (1024, 128)):
                                            nc.tensor.matmul(
                                                ps[:, o:o + n], lhq,
                                                Mt[:, ic, o:o + n],
                                                start=(ic == 0),
                                                stop=(ic == NI - 1))
                                qh = itq.tile([L, PSH], f32, tag="qh")
                                nc.vector.tensor_add(qh[:], ps[:], U_sb[:])
                                qh_cur = qh

                            qm = itq.tile([128, NJ, L], bf16, tag="qm")
                            if it == niters:
                                qo = itq.tile([128, NJ, L], f32, tag="qo")
                            zz = ite.tile([128, NJ], f32, tag="zz")
                            rz = ite.tile([128, NJ], f32, tag="rz")
                            for jc in range(NJ):
                                tp = ittp.tile([128, L], f32, tag="tp")
                                nc.tensor.transpose(
                                    tp[:], qh_cur[:, jc * 128:(jc + 1) * 128],
                                    ident[:])
                                e = ite.tile([128, L], f32, tag="e")
                                nc.scalar.activation(
                                    e[:], tp[:], AF.Exp,
                                    accum_out=zz[:, jc:jc + 1])
                                nc.vector.reciprocal(rz[:, jc:jc + 1],
                                                     zz[:, jc:jc + 1])
                                nc.vector.tensor_scalar_mul(
                                    qm[:, jc, :], e[:], rz[:, jc:jc + 1])
                                if it == niters:
                                    nc.vector.tensor_scalar_mul(
                                        qo[:, jc, :], e[:],
                                        rz[:, jc:jc + 1])
                            if it < niters:
                                nc.gpsimd.dma_start(
                                    qsh_d.rearrange("a b c -> b a c"), qm[:])
                                nc.gpsimd.collective_compute(
                                    "AllGather", mybir.AluOpType.bypass,
                                    replica_groups=rg,
                                    ins=[qsh_d.rearrange("a b c -> (a b c)")],
                                    outs=[qfl_d.rearrange(
                                        "a b c -> (a b c)")])
                                qf = itq.tile([128, NI, L], bf16, tag="qf",
                                              bufs=2)
                                nc.sync.dma_start(
                                    qf[:], qfl_d.rearrange("a b c -> b a c"))
                                qf_cur = qf
                            else:
                                nc.gpsimd.dma_start(
                                    qout[:, :, :].rearrange("a b c -> b a c"),
                                    qo[:])
                mpool.__exit__(None, None, None)

    nc.finalize()
    _CACHE[key] = nc
    return nc


# ----------------------------------------------------------------------------
# host entry point
# ----------------------------------------------------------------------------

def _in_maps(unary, ref):
    s2_blocks, cst, cmat = _host_constants()
    u2 = np.ascontiguousarray(np.asarray(unary, np.float32).reshape(L, P))
    r2 = np.ascontiguousarray(np.asarray(ref, np.float32).reshape(3, P))
    maps = []
    for c in range(NCORES):
        sl = slice(c * PSH, (c + 1) * PSH)
        maps.append({
            "unary_m": np.ascontiguousarray(u2[:, sl]),
            "ref_f": r2,
            "ref_m": np.ascontiguousarray(r2[:, sl]),
            "cst_f": cst,
            "cst_m": np.ascontiguousarray(cst[:, sl]),
            "s2b": s2_blocks[c],
            "cmat": cmat,
        })
    return maps


def kernel(unary: np.ndarray, ref: np.ndarray) -> np.ndarray:
    from concourse import bass_utils

    nc = _build_bass()
    in_maps = _in_maps(unary, ref)

    global LAST_RESULT
    res = bass_utils.run_bass_kernel_spmd(nc, in_maps,
                                          core_ids=list(range(NCORES)),
                                          trace=TRACE)
    LAST_RESULT = res
    shards = [res.results[c]["qout"].reshape(PSH, L) for c in range(NCORES)]
    qfull = np.concatenate(shards, 0)          # [P, L]
    out = qfull.T.reshape(1, L, H, W).astype(np.float32)
    return out


if __name__ == "__main__":
    u = np.random.rand(1, L, H, W).astype(np.float32)
    r = (np.random.rand(1, 3, H, W) * 255).astype(np.float32)
    o = kernel(u, r)
    print(o.shape, o.dtype, o.sum())



# revision 2
# speedup vs baseline: 2.1395x; 2.1395x over previous
"""DenseCRF mean-field inference on 8 Trainium2 NeuronCores — v2.

Math per image: 5 iterations of
    q_hat = U + 4*((q/n) @ K)/n + 2*conv71(q);  q = softmax(q_hat, axis=0)
with K[i,j] = exp(-0.5*d2(i,j)) the dense 9216x9216 bilateral kernel and
n = sqrt(colsum K).

v2 design (vs v1 which folded everything into one rescaled matrix M):
  * Resident matrix is the UNRESCALED E = 4K in fp8e4 (10.6 MB SBUF/core,
    columns sharded).  Built by a 7-row bf16 extended-feature matmul
    (features bf16-rounded on host; diagonal terms computed exactly from
    the rounded features: d_i enters via ACT bias, d_j via hi/lo bf16
    rows, so the quadratic form cancels exactly) + one ACT Exp per chunk
    with accum_out giving partial rowsums for free.
  * rn = 1/sqrt(colsum K): partial rowsums AllGather (bf16, 147KB) +
    local combine (colsum = rowsum by symmetry); per-column rn for my
    shard from a local fp8 DoubleRow ones-matvec colsum (hidden under
    the AllGather).
  * No rescale pass, no dense spatial matrix: rn_i folds into the fp8
    lhsT q-scale (one DVE mult/iter), rn_j into a [21,1152] result
    scale; the 2*conv71 spatial term is computed separably per y-row
    block via tiny band-matrix matmuls (B12 @ Q @ 2B) on the PE.
  * Iteration matvec in fp8 DoubleRow (2 k-tiles/instr, 0.5 cyc/row).
  * Softmax per y-row in transposed [96,21] tiles; AllGather of the
    bf16 q shard (387KB out) per iteration.
"""

import numpy as np
import ml_dtypes

H = 96
W = 96
P = H * W            # 9216 pixels
L = 21               # classes
NCORES = 8
PSH = P // NCORES    # 1152 pixels per core
NI = P // 128        # 72 contraction chunks of 128
NJ = PSH // 128      # 9 (kept for test.py compat)
NR = H // NCORES     # 12 y-rows per core
NPAIR = NI // 2      # 36 DoubleRow pairs
SXY_BF = 70.0
SC_BF = 12.0
SIG_SQ_SP = 36.0
RR_SP = 35
LN4 = float(np.log(4.0))
LN2 = float(np.log(2.0))

_bf16 = ml_dtypes.bfloat16
_fp8 = ml_dtypes.float8_e4m3

_CACHE = {}
TRACE = False
LAST_RESULT = None
NITERS = 5


# ----------------------------------------------------------------------------
# host-side prep
# ----------------------------------------------------------------------------

def _spatial_band():
    if "band" in _CACHE:
        return _CACHE["band"]
    g1 = np.exp(-((np.arange(2 * RR_SP + 1, dtype=np.float64) - RR_SP) ** 2)
                / (2 * SIG_SQ_SP))
    z = g1.sum()
    idx = np.arange(H)
    d = idx[:, None] - idx[None, :]
    B = np.where(np.abs(d) <= RR_SP,
                 np.exp(-(d.astype(np.float64) ** 2) / (2 * SIG_SQ_SP)) / z,
                 0.0).astype(np.float32)
    _CACHE["band"] = B
    return B


def _in_maps(unary, ref):
    B = _spatial_band()
    u = np.asarray(unary, np.float32).reshape(L, P)
    r = np.asarray(ref, np.float32).reshape(3, P)

    ys = (np.arange(P) // W).astype(np.float32)
    xs = (np.arange(P) % W).astype(np.float32)
    f = np.concatenate([ys[None] / SXY_BF, xs[None] / SXY_BF, r / SC_BF], 0)
    fhat = f.astype(_bf16)                       # [5, P] rounded features
    fh32 = fhat.astype(np.float32)
    d = (-0.5 * (fh32 * fh32).sum(0))            # [P] f32, exact from rounded
    dhi = d.astype(_bf16)
    dlo = (d - dhi.astype(np.float32)).astype(_bf16)

    # contraction permutation: chunk n, PE row r <-> pixel 72*r + n, so the
    # per-iteration lhsT gather reads one contiguous 1512B run per partition
    perm = (np.arange(P).reshape(128, NI).T).ravel()   # pos 128*n+r -> 72r+n
    ltP = np.concatenate([fhat, np.ones((2, P), _bf16)], 0)[:, perm]  # [7, P]
    biasE = np.ascontiguousarray(
        (d[perm] + LN4).reshape(NI, 128).T)      # [128, NI] f32

    uc = np.clip(u, 1e-5, 1.0)                   # [L, P]
    Ufull = np.log(uc)
    q0 = uc / uc.sum(0, keepdims=True)           # [L, P]
    q0pm = np.ascontiguousarray(q0.T.astype(_fp8))  # [P, L] pixel-major

    maps = []
    for c in range(NCORES):
        sl = slice(c * PSH, (c + 1) * PSH)
        qrQ = np.concatenate([fhat[:, sl], dhi[None, sl], dlo[None, sl]], 0)
        byM = np.ascontiguousarray(B[c * NR:(c + 1) * NR, :].T.astype(_bf16))
        maps.append({
            "ltP": np.ascontiguousarray(ltP),
            "qrQ": np.ascontiguousarray(qrQ),
            "biasE": biasE,
            "upm": np.ascontiguousarray(
                Ufull[:, sl].T.reshape(NR, W, L).transpose(1, 0, 2)),
            "q0sh": np.ascontiguousarray(q0pm[sl].reshape(NR, W, L)),
            "byM": byM,                                     # [96, 12]
            "bx2": np.ascontiguousarray((2.0 * B).astype(_bf16)),  # [96, 96]
        })
    return maps


# ----------------------------------------------------------------------------
# device program
# ----------------------------------------------------------------------------

def _build_bass(niters=NITERS):
    key = ("nc2", niters)
    if key in _CACHE:
        return _CACHE[key]

    import concourse.bass as bass
    import concourse.bacc as bacc
    import concourse.tile as tile
    import concourse.mybir as mybir
    from concourse.masks import make_identity

    f32 = mybir.dt.float32
    bf16 = mybir.dt.bfloat16
    fp8 = mybir.dt.float8e4
    AF = mybir.ActivationFunctionType
    ALU = mybir.AluOpType
    DR = mybir.MatmulPerfMode.DoubleRow

    nc = bacc.Bacc("TRN2", num_devices=NCORES)

    ltP = nc.dram_tensor("ltP", [7, P], bf16, kind="ExternalInput")
    qrQ = nc.dram_tensor("qrQ", [7, PSH], bf16, kind="ExternalInput")
    biasE = nc.dram_tensor("biasE", [128, NI], f32, kind="ExternalInput")
    upm = nc.dram_tensor("upm", [W, NR, L], f32, kind="ExternalInput")
    q0sh = nc.dram_tensor("q0sh", [NR, W, L], fp8, kind="ExternalInput")
    byM = nc.dram_tensor("byM", [H, NR], bf16, kind="ExternalInput")
    bx2 = nc.dram_tensor("bx2", [W, W], bf16, kind="ExternalInput")
    qout = nc.dram_tensor("qout", [NR, W, L], f32, kind="ExternalOutput")

    rg = [list(range(NCORES))]
    COLS = ((0, 512), (512, 512), (1024, 128))

    with tile.TileContext(nc) as tc:
        with tc.tile_pool(name="dram", bufs=1, space="DRAM") as dram:
            qsh_d = dram.tile([PSH * L], fp8)       # my q shard (r x l)
            qfl_d = dram.tile([P * L], fp8)         # gathered q
            cs_in_d = dram.tile([PSH], bf16)        # my colsums (pixel order)
            rnj_d = dram.tile([1, PSH], f32)        # my rn_j free-major
            cs_out_d = dram.tile([P], bf16)         # all colsums

            qfl_lhs = qfl_d.rearrange("(p n l) -> p n l", p=128, n=NI, l=L)
            qfl_y = qfl_d.rearrange("(y x l) -> y x l", y=H, x=W, l=L)

            with tc.tile_pool(name="persist", bufs=1) as persist:
                ident = persist.tile([L, L], f32)
                make_identity(nc, ident[:])
                ltP_sb = persist.tile([7, P], bf16)
                nc.sync.dma_start(ltP_sb[:], ltP[:, :])
                qrQ_sb = persist.tile([7, PSH], bf16)
                nc.sync.dma_start(qrQ_sb[:], qrQ[:, :])
                biasE_sb = persist.tile([128, NI], f32)
                nc.sync.dma_start(biasE_sb[:], biasE[:, :])
                byM_sb = persist.tile([H, NR], bf16)
                nc.sync.dma_start(byM_sb[:], byM[:, :])
                bx2_sb = persist.tile([W, W], bf16)
                nc.sync.dma_start(bx2_sb[:], bx2[:, :])
                Upm_sb = persist.tile([W, NR, L], f32)
                nc.sync.dma_start(Upm_sb[:], upm[:, :, :])
                ident96b = persist.tile([W, W], bf16)
                make_identity(nc, ident96b[:])
                ident96f = persist.tile([W, W], f32)
                make_identity(nc, ident96f[:])

                Mt = persist.tile([128, NI, PSH], fp8, name="Mt")
                rnIsp = persist.tile([128, NI, L], bf16)
                rnJf = persist.tile([1, PSH], f32)
                rnJ21 = persist.tile([L, PSH], f32)
                ones2 = persist.tile([128, 2, 32], fp8)
                nc.vector.memset(ones2[:], 1.0)
                ln2c = persist.tile([128, 1], f32)
                nc.vector.memset(ln2c[:], LN2)
                qs = persist.tile([128, NI, 32], fp8, name="qs")
                nc.vector.memset(qs[:], 0.0)

                # ---- q0 AllGather (overlaps the E build) ------------------
                nc.gpsimd.dma_start(
                    qsh_d.rearrange("(r x l) -> r x l", r=NR, x=W, l=L),
                    q0sh[:, :, :])
                nc.gpsimd.collective_compute(
                    "AllGather", mybir.AluOpType.bypass, replica_groups=rg,
                    ins=[qsh_d[:]], outs=[qfl_d[:]])

                # ---- E = exp(T') build + local colsum ---------------------
                csb = persist.tile([1, PSH], f32)
                with (
                    tc.tile_pool(name="eps", bufs=2, space="PSUM") as eps,
                    tc.tile_pool(name="csp", bufs=1, space="PSUM") as csp,
                ):
                    csA = csp.tile([32, 512], f32, name="csA")
                    csB = csp.tile([32, 512], f32, name="csB")
                    for ic in range(NI):
                        ps = eps.tile([128, PSH], f32, tag="eps")
                        lh = ltP_sb[:, ic * 128:(ic + 1) * 128]
                        for (o, n) in COLS:
                            nc.tensor.matmul(ps[:, o:o + n], lh,
                                             qrQ_sb[:, o:o + n],
                                             start=True, stop=True)
                        nc.scalar.activation(
                            Mt[:, ic, :], ps[:], AF.Exp,
                            bias=biasE_sb[:, ic:ic + 1])
                        if ic % 2 == 1:
                            # colsum ranges A/B accumulate as pairs complete
                            k = ic // 2
                            for cs_t, o in ((csA, 0), (csB, 512)):
                                nc.tensor.matmul(
                                    cs_t[:], ones2[:],
                                    Mt[:, ic - 1:ic + 1, o:o + 512],
                                    start=(k == 0), stop=(k == NPAIR - 1),
                                    perf_mode=DR, skip_group_check=True)
                    nc.vector.tensor_copy(csb[0:1, 0:512], csA[0:1, :])
                    nc.vector.tensor_copy(csb[0:1, 512:1024], csB[0:1, :])

                # colsum range C (post-build) + rn_j
                with tc.tile_pool(name="cspC", bufs=1, space="PSUM") as cspC:
                    csC = cspC.tile([32, 128], f32, name="csC")
                    for k in range(NPAIR):
                        nc.tensor.matmul(
                            csC[:], ones2[:],
                            Mt[:, 2 * k:2 * k + 2, 1024:1152],
                            start=(k == 0), stop=(k == NPAIR - 1),
                            perf_mode=DR)
                    nc.vector.tensor_copy(csb[0:1, 1024:1152], csC[0:1, :])

                    # rn_j for my columns (free-major) from local colsum
                    lcj = persist.tile([1, PSH], f32)
                    nc.scalar.activation(lcj[:], csb[0:1, :], AF.Ln)
                    nc.scalar.activation(rnJf[:], lcj[:], AF.Exp,
                                         bias=ln2c[0:1, :], scale=-0.5)
                    nc.gpsimd.dma_start(rnj_d[:, :], rnJf[:])
                    nc.sync.dma_start(
                        rnJ21[:], rnj_d[0:1, :].to_broadcast((L, PSH)))
                    csbh = persist.tile([1, PSH], bf16)
                    nc.vector.tensor_copy(csbh[:], csb[:])
                    nc.gpsimd.dma_start(
                        cs_in_d.rearrange("(a q) -> a q", a=1, q=PSH),
                        csbh[:])

                # ---- rn_i for all pixels via colsum AllGather -------------
                nc.gpsimd.collective_compute(
                    "AllGather", mybir.AluOpType.bypass, replica_groups=rg,
                    ins=[cs_in_d[:]], outs=[cs_out_d[:]])
                csg = persist.tile([128, NI], bf16)
                nc.sync.dma_start(
                    csg[:], cs_out_d.rearrange("(p n) -> p n", p=128, n=NI))
                lci = persist.tile([128, NI], f32)
                nc.scalar.activation(lci[:], csg[:], AF.Ln)
                rnf = persist.tile([128, NI], f32)
                nc.scalar.activation(rnf[:], lci[:], AF.Exp,
                                     bias=ln2c[:, :], scale=-0.5)
                nc.vector.tensor_copy(
                    rnIsp[:], rnf[:].to_broadcast((128, NI, L)))

                # ---- iterations ------------------------------------------
                with (
                    tc.tile_pool(name="itq", bufs=1) as itq,
                    tc.tile_pool(name="mmp", bufs=1, space="PSUM") as mmp,
                    tc.tile_pool(name="spp", bufs=1, space="PSUM") as spp,
                    tc.tile_pool(name="tpp", bufs=1, space="PSUM") as tpp,
                ):
                    for it in range(1, niters + 1):
                        qf = itq.tile([128, NI, L], fp8, tag="qf", bufs=2)
                        nc.sync.dma_start(qf[:], qfl_lhs)
                        qy = itq.tile([H, W, L], fp8, tag="qy", bufs=2)
                        nc.sync.dma_start(qy[:], qfl_y)

                        # lhsT = q * rn_i in fp8 (pad cols 21:32 stay 0)
                        nc.vector.tensor_mul(qs[:, 0:NI // 2, 0:L],
                                             qf[:, 0:NI // 2, :],
                                             rnIsp[:, 0:NI // 2, :])
                        nc.vector.tensor_mul(qs[:, NI // 2:, 0:L],
                                             qf[:, NI // 2:, :],
                                             rnIsp[:, NI // 2:, :])

                        # spatial: tmpY = By_mine^T @ Qy  -> [12, (x l)]
                        tmpYs = itq.tile([NR, W, L], f32, tag="tmpYs")
                        for h in range(2):
                            tmpY = spp.tile([NR, 2, 512], f32, tag="tmpY")
                            for g2 in range(2):
                                g = 2 * h + g2
                                nc.tensor.matmul(
                                    tmpY[:, g2, 0:504], byM_sb[:],
                                    qy[:, 24 * g:24 * (g + 1), :],
                                    start=True, stop=True)
                            nc.scalar.copy(tmpYs[:, 48 * h:48 * (h + 1), :],
                                           tmpY[:, :, 0:504])
                        txa = spp.tile([W, L, NR], f32, tag="txa")
                        for ll in range(L):
                            nc.tensor.transpose(txa[:, ll, :],
                                                tmpYs[:, :, ll],
                                                ident[0:NR, 0:NR])
                        txs = itq.tile([W, L, NR], bf16, tag="txs")
                        nc.scalar.copy(txs[:], txa[:])
                        qsfT = spp.tile([W, L, NR], f32, tag="txa")
                        nc.tensor.matmul(qsfT[:], bx2_sb[:], txs[:],
                                         start=True, stop=True)
                        qsfS = itq.tile([W, L, NR], bf16, tag="qsfS")
                        nc.scalar.copy(qsfS[:], qsfT[:])

                        # bilateral: ps = (q rn) @ E, fp8 DoubleRow.
                        # Range-major so qh/softmax of early columns overlap
                        # the later ranges' accumulation.
                        ps = mmp.tile([32, PSH], f32, tag="ps")
                        qh = itq.tile([L, PSH], f32, tag="qh")
                        for (o, n) in COLS:
                            for k in range(NPAIR):
                                nc.tensor.matmul(
                                    ps[:, o:o + n], qs[:, 2 * k:2 * k + 2, :],
                                    Mt[:, 2 * k:2 * k + 2, o:o + n],
                                    start=(k == 0), stop=(k == NPAIR - 1),
                                    perf_mode=DR)
                            nc.vector.tensor_mul(qh[:, o:o + n],
                                                 ps[0:L, o:o + n],
                                                 rnJ21[:, o:o + n])

                        # per-y-row transpose; spatial + U folded in via
                        # identity-matmul accumulation on the PE
                        tp = tpp.tile([W, NR, L], f32, tag="tp")
                        for r in range(NR):
                            nc.tensor.matmul(
                                tp[:, r, :], qh[:, r * W:(r + 1) * W],
                                ident[:], is_transpose=True,
                                start=True, stop=False)
                            nc.tensor.matmul(
                                tp[:, r, :], ident96b[:], qsfS[:, :, r],
                                start=False, stop=False)
                            nc.tensor.matmul(
                                tp[:, r, :], ident96f[:], Upm_sb[:, r, :],
                                start=False, stop=True)
                        e = itq.tile([W, NR, L], f32, tag="e")
                        nc.scalar.activation(e[:], tp[:], AF.Exp)
                        zz = itq.tile([W, NR], f32, tag="zz")
                        nc.vector.tensor_reduce(zz[:], e[:],
                                                mybir.AxisListType.X,
                                                ALU.add)
                        rz = itq.tile([W, NR], f32, tag="rz")
                        nc.vector.reciprocal(rz[:], zz[:])
                        if it < niters:
                            qm = itq.tile([W, NR, L], fp8, tag="qm")
                            nc.vector.tensor_mul(
                                qm[:], e[:],
                                rz[:].to_broadcast((W, NR, L)))
                            nc.sync.dma_start(
                                qsh_d.rearrange("(r x l) -> x r l",
                                                r=NR, x=W, l=L), qm[:])
                            nc.gpsimd.collective_compute(
                                "AllGather", mybir.AluOpType.bypass,
                                replica_groups=rg,
                                ins=[qsh_d[:]], outs=[qfl_d[:]])
                        else:
                            qo = itq.tile([W, NR, L], f32, tag="qo")
                            nc.vector.tensor_mul(
                                qo[:], e[:],
                                rz[:].to_broadcast((W, NR, L)))
                            nc.sync.dma_start(
                                qout[:, :, :].rearrange("r x l -> x r l"),
                                qo[:])

    nc.finalize()
    _CACHE[key] = nc
    return nc


# ----------------------------------------------------------------------------
# host entry point
# ----------------------------------------------------------------------------

def kernel(unary: np.ndarray, ref: np.ndarray) -> np.ndarray:
    from concourse import bass_utils

    nc = _build_bass()
    in_maps = _in_maps(unary, ref)

    global LAST_RESULT
    res = bass_utils.run_bass_kernel_spmd(nc, in_maps,
                                          core_ids=list(range(NCORES)),
                                          trace=TRACE)
    LAST_RESULT = res
    shards = [res.results[c]["qout"].reshape(PSH, L) for c in range(NCORES)]
    qfull = np.concatenate(shards, 0)          # [P, L]
    out = qfull.T.reshape(1, L, H, W).astype(np.float32)
    return out


if __name__ == "__main__":
    u = np.random.rand(1, L, H, W).astype(np.float32)
    r = (np.random.rand(1, 3, H, W) * 255).astype(np.float32)
    o = kernel(u, r)
    print(o.shape, o.dtype, o.sum())


# revision 3
# speedup vs baseline: 4.3045x; 2.0120x over previous
"""DenseCRF mean-field inference on 8 Trainium2 NeuronCores — v2.

Math per image: 5 iterations of
    q_hat = U + 4*((q/n) @ K)/n + 2*conv71(q);  q = softmax(q_hat, axis=0)
with K[i,j] = exp(-0.5*d2(i,j)) the dense 9216x9216 bilateral kernel and
n = sqrt(colsum K).

v2 design (vs v1 which folded everything into one rescaled matrix M):
  * Resident matrix is the UNRESCALED E = 4K in fp8e4 (10.6 MB SBUF/core,
    columns sharded).  Built by a 7-row bf16 extended-feature matmul
    (features bf16-rounded on host; diagonal terms computed exactly from
    the rounded features: d_i enters via ACT bias, d_j via hi/lo bf16
    rows, so the quadratic form cancels exactly) + one ACT Exp per chunk
    with accum_out giving partial rowsums for free.
  * rn = 1/sqrt(colsum K): partial rowsums AllGather (bf16, 147KB) +
    local combine (colsum = rowsum by symmetry); per-column rn for my
    shard from a local fp8 DoubleRow ones-matvec colsum (hidden under
    the AllGather).
  * No rescale pass, no dense spatial matrix: rn_i folds into the fp8
    lhsT q-scale (one DVE mult/iter), rn_j into a [21,1152] result
    scale; the 2*conv71 spatial term is computed separably per y-row
    block via tiny band-matrix matmuls (B12 @ Q @ 2B) on the PE.
  * Iteration matvec in fp8 DoubleRow (2 k-tiles/instr, 0.5 cyc/row).
  * Softmax per y-row in transposed [96,21] tiles; AllGather of the
    bf16 q shard (387KB out) per iteration.
"""

import numpy as np
import ml_dtypes

H = 96
W = 96
P = H * W            # 9216 pixels
L = 21               # classes
NCORES = 8
PSH = P // NCORES    # 1152 pixels per core
NI = P // 128        # 72 contraction chunks of 128
NJ = PSH // 128      # 9 (kept for test.py compat)
NR = H // NCORES     # 12 y-rows per core
NPAIR = NI // 2      # 36 DoubleRow pairs
SXY_BF = 70.0
SC_BF = 12.0
SIG_SQ_SP = 36.0
RR_SP = 35
LN4 = float(np.log(4.0))
LN2 = float(np.log(2.0))

_bf16 = ml_dtypes.bfloat16
_fp8 = ml_dtypes.float8_e4m3

_CACHE = {}
TRACE = False
LAST_RESULT = None
NITERS = 5


# ----------------------------------------------------------------------------
# host-side prep
# ----------------------------------------------------------------------------

def _spatial_band():
    if "band" in _CACHE:
        return _CACHE["band"]
    g1 = np.exp(-((np.arange(2 * RR_SP + 1, dtype=np.float64) - RR_SP) ** 2)
                / (2 * SIG_SQ_SP))
    z = g1.sum()
    idx = np.arange(H)
    d = idx[:, None] - idx[None, :]
    B = np.where(np.abs(d) <= RR_SP,
                 np.exp(-(d.astype(np.float64) ** 2) / (2 * SIG_SQ_SP)) / z,
                 0.0).astype(np.float32)
    _CACHE["band"] = B
    return B


def _in_maps(unary, ref):
    B = _spatial_band()
    u = np.asarray(unary, np.float32).reshape(L, P)
    r = np.asarray(ref, np.float32).reshape(3, P)

    ys = (np.arange(P) // W).astype(np.float32)
    xs = (np.arange(P) % W).astype(np.float32)
    f = np.concatenate([ys[None] / SXY_BF, xs[None] / SXY_BF, r / SC_BF], 0)
    fhat = f.astype(_bf16)                       # [5, P] rounded features
    fh32 = fhat.astype(np.float32)
    d = (-0.5 * (fh32 * fh32).sum(0))            # [P] f32, exact from rounded
    dhi = d.astype(_bf16)
    dlo = (d - dhi.astype(np.float32)).astype(_bf16)

    # contraction permutation: chunk n, PE row r <-> pixel 72*r + n, so the
    # per-iteration lhsT gather reads one contiguous 1512B run per partition
    perm = (np.arange(P).reshape(128, NI).T).ravel()   # pos 128*n+r -> 72r+n
    ltP = np.concatenate([fhat, np.ones((2, P), _bf16)], 0)[:, perm]  # [7, P]
    biasE = np.ascontiguousarray(
        (d[perm] + LN4).reshape(NI, 128).T)      # [128, NI] f32

    uc = np.clip(u, 1e-5, 1.0)                   # [L, P]
    Ufull = np.log(uc)
    q0 = uc / uc.sum(0, keepdims=True)           # [L, P]
    q0pm = np.ascontiguousarray(q0.T.astype(_fp8))  # [P, L] pixel-major

    maps = []
    for c in range(NCORES):
        sl = slice(c * PSH, (c + 1) * PSH)
        qrQ = np.concatenate([fhat[:, sl], dhi[None, sl], dlo[None, sl]], 0)
        byM = np.ascontiguousarray(B[c * NR:(c + 1) * NR, :].T.astype(_bf16))
        maps.append({
            "ltP": np.ascontiguousarray(ltP),
            "qrQ": np.ascontiguousarray(qrQ),
            "biasE": biasE,
            "upm": np.ascontiguousarray(
                Ufull[:, sl].T.reshape(NR, W, L).transpose(1, 0, 2)),
            "q0sh": np.ascontiguousarray(q0pm[sl].reshape(NR, W, L)),
            "byM": byM,                                     # [96, 12]
            "bx2": np.ascontiguousarray((2.0 * B).astype(_bf16)),  # [96, 96]
        })
    return maps


# ----------------------------------------------------------------------------
# device program
# ----------------------------------------------------------------------------

def _build_bass(niters=NITERS):
    key = ("nc2", niters)
    if key in _CACHE:
        return _CACHE[key]

    import concourse.bass as bass
    import concourse.bacc as bacc
    import concourse.tile as tile
    import concourse.mybir as mybir
    from concourse.masks import make_identity

    f32 = mybir.dt.float32
    bf16 = mybir.dt.bfloat16
    fp8 = mybir.dt.float8e4
    AF = mybir.ActivationFunctionType
    ALU = mybir.AluOpType
    DR = mybir.MatmulPerfMode.DoubleRow

    nc = bacc.Bacc("TRN2", num_devices=NCORES)

    ltP = nc.dram_tensor("ltP", [7, P], bf16, kind="ExternalInput")
    qrQ = nc.dram_tensor("qrQ", [7, PSH], bf16, kind="ExternalInput")
    biasE = nc.dram_tensor("biasE", [128, NI], f32, kind="ExternalInput")
    upm = nc.dram_tensor("upm", [W, NR, L], f32, kind="ExternalInput")
    q0sh = nc.dram_tensor("q0sh", [NR, W, L], fp8, kind="ExternalInput")
    byM = nc.dram_tensor("byM", [H, NR], bf16, kind="ExternalInput")
    bx2 = nc.dram_tensor("bx2", [W, W], bf16, kind="ExternalInput")
    qout = nc.dram_tensor("qout", [NR, W, L], f32, kind="ExternalOutput")

    rg = [list(range(NCORES))]
    COLS = ((0, 512), (512, 512), (1024, 128))

    with tile.TileContext(nc) as tc:
        with tc.tile_pool(name="dram", bufs=1, space="DRAM") as dram:
            qsh_d = dram.tile([PSH * L], fp8)       # my q shard (r x l)
            qfl_d = dram.tile([P * L], fp8)         # gathered q
            cs_in_d = dram.tile([PSH], bf16)        # my colsums (pixel order)
            rnj_d = dram.tile([1, PSH], f32)        # my rn_j free-major
            cs_out_d = dram.tile([P], bf16)         # all colsums

            qfl_lhs = qfl_d.rearrange("(p n l) -> p n l", p=128, n=NI, l=L)
            qfl_y = qfl_d.rearrange("(y x l) -> y x l", y=H, x=W, l=L)

            with tc.tile_pool(name="persist", bufs=1) as persist:
                ident = persist.tile([L, L], f32)
                make_identity(nc, ident[:])
                ltP_sb = persist.tile([7, P], bf16)
                nc.sync.dma_start(ltP_sb[:], ltP[:, :])
                qrQ_sb = persist.tile([7, PSH], bf16)
                nc.sync.dma_start(qrQ_sb[:], qrQ[:, :])
                biasE_sb = persist.tile([128, NI], f32)
                nc.sync.dma_start(biasE_sb[:], biasE[:, :])
                byM_sb = persist.tile([H, NR], bf16)
                nc.sync.dma_start(byM_sb[:], byM[:, :])
                bx2_sb = persist.tile([W, W], bf16)
                nc.sync.dma_start(bx2_sb[:], bx2[:, :])
                Upm_sb = persist.tile([W, NR, L], f32)
                nc.sync.dma_start(Upm_sb[:], upm[:, :, :])
                ident96b = persist.tile([W, W], bf16)
                make_identity(nc, ident96b[:])
                ident96f = persist.tile([W, W], f32)
                make_identity(nc, ident96f[:])

                Mt = persist.tile([128, NI, PSH], fp8, name="Mt")
                rnIsp = persist.tile([128, NI, L], bf16)
                rnJf = persist.tile([1, PSH], f32)
                rnJ21 = persist.tile([L, PSH], f32)
                ones2 = persist.tile([128, 2, 32], fp8)
                nc.vector.memset(ones2[:], 1.0)
                ln2c = persist.tile([128, 1], f32)
                nc.vector.memset(ln2c[:], LN2)
                qs = persist.tile([128, NI, 32], fp8, name="qs")
                nc.vector.memset(qs[:], 0.0)

                # ---- q0 AllGather (overlaps the E build) ------------------
                nc.gpsimd.dma_start(
                    qsh_d.rearrange("(r x l) -> r x l", r=NR, x=W, l=L),
                    q0sh[:, :, :])
                nc.gpsimd.collective_compute(
                    "AllGather", mybir.AluOpType.bypass, replica_groups=rg,
                    ins=[qsh_d[:]], outs=[qfl_d[:]])

                # ---- E = exp(T') build + local colsum ---------------------
                csb = persist.tile([1, PSH], f32)
                with (
                    tc.tile_pool(name="eps", bufs=2, space="PSUM") as eps,
                    tc.tile_pool(name="csp", bufs=1, space="PSUM") as csp,
                ):
                    csA = csp.tile([32, 512], f32, name="csA")
                    csB = csp.tile([32, 512], f32, name="csB")
                    for ic in range(NI):
                        ps = eps.tile([128, PSH], f32, tag="eps")
                        lh = ltP_sb[:, ic * 128:(ic + 1) * 128]
                        for (o, n) in COLS:
                            nc.tensor.matmul(ps[:, o:o + n], lh,
                                             qrQ_sb[:, o:o + n],
                                             start=True, stop=True)
                        nc.scalar.activation(
                            Mt[:, ic, :], ps[:], AF.Exp,
                            bias=biasE_sb[:, ic:ic + 1])
                        if ic % 2 == 1:
                            # colsum ranges A/B accumulate as pairs complete
                            k = ic // 2
                            for cs_t, o in ((csA, 0), (csB, 512)):
                                nc.tensor.matmul(
                                    cs_t[:], ones2[:],
                                    Mt[:, ic - 1:ic + 1, o:o + 512],
                                    start=(k == 0), stop=(k == NPAIR - 1),
                                    perf_mode=DR, skip_group_check=True)
                    nc.vector.tensor_copy(csb[0:1, 0:512], csA[0:1, :])
                    nc.vector.tensor_copy(csb[0:1, 512:1024], csB[0:1, :])

                # colsum range C (post-build) + rn_j
                with tc.tile_pool(name="cspC", bufs=1, space="PSUM") as cspC:
                    csC = cspC.tile([32, 128], f32, name="csC")
                    for k in range(NPAIR):
                        nc.tensor.matmul(
                            csC[:], ones2[:],
                            Mt[:, 2 * k:2 * k + 2, 1024:1152],
                            start=(k == 0), stop=(k == NPAIR - 1),
                            perf_mode=DR)
                    nc.vector.tensor_copy(csb[0:1, 1024:1152], csC[0:1, :])

                    # rn_j for my columns (free-major) from local colsum
                    lcj = persist.tile([1, PSH], f32)
                    nc.scalar.activation(lcj[:], csb[0:1, :], AF.Ln)
                    nc.scalar.activation(rnJf[:], lcj[:], AF.Exp,
                                         bias=ln2c[0:1, :], scale=-0.5)
                    nc.gpsimd.dma_start(rnj_d[:, :], rnJf[:])
                    nc.sync.dma_start(
                        rnJ21[:], rnj_d[0:1, :].to_broadcast((L, PSH)))
                    csbh = persist.tile([1, PSH], bf16)
                    nc.vector.tensor_copy(csbh[:], csb[:])
                    nc.gpsimd.dma_start(
                        cs_in_d.rearrange("(a q) -> a q", a=1, q=PSH),
                        csbh[:])

                # ---- rn_i for all pixels via colsum AllGather -------------
                nc.gpsimd.collective_compute(
                    "AllGather", mybir.AluOpType.bypass, replica_groups=rg,
                    ins=[cs_in_d[:]], outs=[cs_out_d[:]])
                csg = persist.tile([128, NI], bf16)
                nc.sync.dma_start(
                    csg[:], cs_out_d.rearrange("(p n) -> p n", p=128, n=NI))
                lci = persist.tile([128, NI], f32)
                nc.scalar.activation(lci[:], csg[:], AF.Ln)
                rnf = persist.tile([128, NI], f32)
                nc.scalar.activation(rnf[:], lci[:], AF.Exp,
                                     bias=ln2c[:, :], scale=-0.5)
                nc.vector.tensor_copy(
                    rnIsp[:], rnf[:].to_broadcast((128, NI, L)))

                # ---- iterations ------------------------------------------
                with (
                    tc.tile_pool(name="itq", bufs=1) as itq,
                    tc.tile_pool(name="mmp", bufs=1, space="PSUM") as mmp,
                    tc.tile_pool(name="spp", bufs=1, space="PSUM") as spp,
                    tc.tile_pool(name="tpp", bufs=1, space="PSUM") as tpp,
                    tc.tile_pool(name="wmp", bufs=1, space="PSUM") as wmp,
                ):
                    def pe_warm(nwm):
                        # keep the PE p-state hot through a collective:
                        # f32 matmuls (4 cyc/row) into a scrap bank
                        wt = wmp.tile([W, NR, L], f32, tag="warm")
                        for _ in range(nwm):
                            nc.tensor.matmul(wt[:], ident96f[:],
                                             Upm_sb[:, :, :],
                                             start=True, stop=True)

                    pe_warm(26)
                    for it in range(1, niters + 1):
                        qf = itq.tile([128, NI, L], fp8, tag="qf", bufs=2)
                        nc.sync.dma_start(qf[:], qfl_lhs)
                        qy = itq.tile([H, W, L], fp8, tag="qy", bufs=2)
                        nc.scalar.dma_start(qy[:], qfl_y)

                        # lhsT = q * rn_i in fp8 (pad cols 21:32 stay 0)
                        nc.vector.tensor_mul(qs[:, 0:NI // 2, 0:L],
                                             qf[:, 0:NI // 2, :],
                                             rnIsp[:, 0:NI // 2, :])
                        nc.vector.tensor_mul(qs[:, NI // 2:, 0:L],
                                             qf[:, NI // 2:, :],
                                             rnIsp[:, NI // 2:, :])

                        # spatial: tmpY = By_mine^T @ Qy  -> [12, (x l)]
                        tmpYs = itq.tile([NR, W, L], f32, tag="tmpYs")
                        for h in range(2):
                            tmpY = spp.tile([NR, 2, 512], f32, tag="tmpY")
                            for g2 in range(2):
                                g = 2 * h + g2
                                nc.tensor.matmul(
                                    tmpY[:, g2, 0:504], byM_sb[:],
                                    qy[:, 24 * g:24 * (g + 1), :],
                                    start=True, stop=True)
                            nc.scalar.copy(tmpYs[:, 48 * h:48 * (h + 1), :],
                                           tmpY[:, :, 0:504])
                        txa = spp.tile([W, L, NR], f32, tag="txa")
                        for ll in range(L):
                            nc.tensor.transpose(txa[:, ll, :],
                                                tmpYs[:, :, ll],
                                                ident[0:NR, 0:NR])
                        txs = itq.tile([W, L, NR], bf16, tag="txs")
                        nc.scalar.copy(txs[:], txa[:])
                        qsfT = spp.tile([W, L, NR], f32, tag="txa")
                        nc.tensor.matmul(qsfT[:], bx2_sb[:], txs[:],
                                         start=True, stop=True)
                        qsfS = itq.tile([W, L, NR], bf16, tag="qsfS")
                        nc.scalar.copy(qsfS[:], qsfT[:])

                        # bilateral: ps = (q rn) @ E, fp8 DoubleRow.
                        # Range-major so qh/softmax of early columns overlap
                        # the later ranges' accumulation.
                        ps = mmp.tile([32, PSH], f32, tag="ps")
                        qh = itq.tile([L, PSH], f32, tag="qh")
                        for (o, n) in COLS:
                            for k in range(NPAIR):
                                nc.tensor.matmul(
                                    ps[:, o:o + n], qs[:, 2 * k:2 * k + 2, :],
                                    Mt[:, 2 * k:2 * k + 2, o:o + n],
                                    start=(k == 0), stop=(k == NPAIR - 1),
                                    perf_mode=DR)
                            nc.vector.tensor_mul(qh[:, o:o + n],
                                                 ps[0:L, o:o + n],
                                                 rnJ21[:, o:o + n])

                        # per-y-row transpose; spatial + U folded in via
                        # identity-matmul accumulation on the PE
                        tp = tpp.tile([W, NR, L], f32, tag="tp")
                        for r in range(NR):
                            nc.tensor.matmul(
                                tp[:, r, :], qh[:, r * W:(r + 1) * W],
                                ident[:], is_transpose=True,
                                start=True, stop=False)
                            nc.tensor.matmul(
                                tp[:, r, :], ident96b[:], qsfS[:, :, r],
                                start=False, stop=False)
                            nc.tensor.matmul(
                                tp[:, r, :], ident96f[:], Upm_sb[:, r, :],
                                start=False, stop=True)
                        e = itq.tile([W, NR, L], f32, tag="e")
                        nc.scalar.activation(e[:], tp[:], AF.Exp)
                        zz = itq.tile([W, NR], f32, tag="zz")
                        nc.vector.tensor_reduce(zz[:], e[:],
                                                mybir.AxisListType.X,
                                                ALU.add)
                        rz = itq.tile([W, NR], f32, tag="rz")
                        nc.vector.reciprocal(rz[:], zz[:])
                        if it < niters:
                            qm = itq.tile([W, NR, L], fp8, tag="qm")
                            nc.vector.tensor_mul(
                                qm[:], e[:],
                                rz[:].to_broadcast((W, NR, L)))
                            nc.sync.dma_start(
                                qsh_d.rearrange("(r x l) -> x r l",
                                                r=NR, x=W, l=L), qm[:])
                            nc.gpsimd.collective_compute(
                                "AllGather", mybir.AluOpType.bypass,
                                replica_groups=rg,
                                ins=[qsh_d[:]], outs=[qfl_d[:]])
                            pe_warm(29)
                        else:
                            qo = itq.tile([W, NR, L], f32, tag="qo")
                            nc.vector.tensor_mul(
                                qo[:], e[:],
                                rz[:].to_broadcast((W, NR, L)))
                            nc.sync.dma_start(
                                qout[:, :, :].rearrange("r x l -> x r l"),
                                qo[:])

    nc.finalize()
    _CACHE[key] = nc
    return nc


# ----------------------------------------------------------------------------
# host entry point
# ----------------------------------------------------------------------------

def kernel(unary: np.ndarray, ref: np.ndarray) -> np.ndarray:
    from concourse import bass_utils

    nc = _build_bass()
    in_maps = _in_maps(unary, ref)

    global LAST_RESULT
    res = bass_utils.run_bass_kernel_spmd(nc, in_maps,
                                          core_ids=list(range(NCORES)),
                                          trace=TRACE)
    LAST_RESULT = res
    shards = [res.results[c]["qout"].reshape(PSH, L) for c in range(NCORES)]
    qfull = np.concatenate(shards, 0)          # [P, L]
    out = qfull.T.reshape(1, L, H, W).astype(np.float32)
    return out


if __name__ == "__main__":
    u = np.random.rand(1, L, H, W).astype(np.float32)
    r = (np.random.rand(1, 3, H, W) * 255).astype(np.float32)
    o = kernel(u, r)
    print(o.shape, o.dtype, o.sum())
